# revision 1
# baseline (speedup 1.0000x reference)
"""Weighted two-sided chamfer loss (AutoDecLoss) for Trainium2 -- 8 cores.

Same proven pipeline as the bf16-split baseline (per block: fp PE matmuls
-> ACT copies one PSUM tile to SBUF -> DVE MIN_MIN_REDUCE pairs it with
the other PSUM tile), but the distance matmul runs in fp32r with plain
K=9 features

    d[n, m] = sum_k X[k, n] * Y[k, m],
    X = [x^2, -2x, 1] rows, Y = [1, y, y^2] rows,

which deletes the entire bf16 hi/lo compensation setup (the DVE/Pool
feature-split chains of the baseline).  fp32r streams 1 row/cycle at
moving-dim 512, same PE cost as bf16.
"""

import re

import numpy as np

import concourse.bacc as bacc
import concourse.mybir as mybir
import concourse.tile as tile
from concourse import dve_ops, masks
from concourse.bass_utils import run_bass_kernel_spmd
from concourse.dve_spec import C0, Spec, Src0, Src1, minn
from concourse.dve_table_gen import dve_ver_for


_OP_NAME = "MIN_MIN_REDUCE_ANT"


def _ref(in0, in1, s0, s1, imm2):
    out = np.minimum(in0.astype(np.float32), in1.astype(np.float32))
    P = out.shape[0]
    body = out.reshape(P, -1)
    seed = np.asarray(s0, np.float32).reshape(-1, 1)
    acc = np.minimum(np.minimum.reduce(body, axis=-1, keepdims=True), seed)
    return out, acc


def get_min_min_reduce():
    for op in dve_ops.OPS:
        if op.name == _OP_NAME:
            return op
    spec = Spec(body=minn(Src0, Src1), accum=minn, accum_init=C0, reference=_ref)
    ver = dve_ver_for("TRN2")
    probe = dve_ops.DveOp(_OP_NAME, spec, subdim=False, uops_sha={})
    row = dve_ops._CUSTOM_DVE_ROW_BASE + len(dve_ops.OPS)
    dve_ops._SUB_OPCODE_FOR_NAME[_OP_NAME] = row
    shas = {}
    for v in ("v3", "v4"):
        try:
            probe.compile(v)
            shas[v] = probe.uops_sha.get(v)
        except ValueError as e:
            m = re.search(rf"{v}: ([0-9a-f]+)", str(e))
            if not m:
                raise
            shas[v] = m.group(1)
    op = dve_ops.DveOp(_OP_NAME, spec, subdim=False, uops_sha=shas)
    dve_ops.OPS.append(op)
    dve_ops.CUSTOM_DVE_SPECS[_OP_NAME] = spec
    assert dve_ops.get_dve_sub_opcode(_OP_NAME) == row
    assert row < 0x20
    assert ver in shas
    return op


def min_min_reduce(nc, out, in0, in1, init, accum_out):
    op = get_min_min_reduce()
    return nc.vector._custom_dve(op, out=out, in0=in0, in1=in1, s0=init,
                                 accum_out=accum_out)


B, N, M = 8, 2048, 4096
NT = N // 128
MT = M // 128
CHAMFER_EPS = 1e-6
MIN_BW = 1e-3
BIG = 3.0e38

F32 = mybir.dt.float32
F32R = mybir.dt.float32r
MIN = mybir.AluOpType.min
ADD = mybir.AluOpType.add
MULT = mybir.AluOpType.mult
AX = mybir.AxisListType.X


def build_nc():
    nc = bacc.Bacc("TRN2", target_bir_lowering=False, debug=False, num_devices=8)
    xT = nc.dram_tensor("xT", [3, N], F32, kind="ExternalInput")
    yT = nc.dram_tensor("yT", [3, M], F32R, kind="ExternalInput")
    wT = nc.dram_tensor("wT", [128, NT], F32, kind="ExternalInput")
    ones3 = nc.dram_tensor("ones3", [3, M], F32R, kind="ExternalInput")
    out = nc.dram_tensor("loss", [1, 1], F32, kind="ExternalOutput")

    with tile.TileContext(nc) as tc:
        with (
            tc.tile_pool(name="feat", bufs=1) as fpool,
            tc.tile_pool(name="small", bufs=1) as spool,
        ):
            # ---------------- feature tiles ----------------
            X9 = fpool.tile([9, N], F32R, tag="X9")
            Y9 = fpool.tile([9, M], F32R, tag="Y9")
            XS9 = fpool.tile([9, N], F32R, tag="XS9")
            xr = fpool.tile([3, N], F32, tag="xr")
            yr = fpool.tile([3, M], F32, tag="yr")
            xsq = fpool.tile([3, N], F32R, tag="xsq")
            xm2 = fpool.tile([3, N], F32R, tag="xm2")
            ysq = fpool.tile([3, M], F32R, tag="ysq")

            wN = spool.tile([128, NT], F32, tag="wN")
            identf = spool.tile([128, 128], F32, tag="identf")
            ones1 = spool.tile([1, 9], F32R, tag="ones1")

            # ---------------- lead-in DMAs ----------------
            nc.sync.dma_start(xr[:], xT[:])
            nc.sync.dma_start(yr[:], yT[:].bitcast(F32))
            nc.sync.dma_start(Y9[3:6, :], yT[:])
            nc.sync.dma_start(wN[:], wT[:])
            nc.scalar.dma_start(Y9[0:3, :], ones3[:])
            nc.scalar.dma_start(X9[6:9, :], ones3[:, 0:N])
            nc.scalar.dma_start(ones1[:], ones3[0:1, 0:9])

            # ---------------- w / r chain (DVE, tiny) ----------------
            wc = spool.tile([128, NT], F32, tag="wc")
            nc.vector.tensor_scalar_max(wc[:], wN[:], MIN_BW)
            rw = spool.tile([128, NT], F32, tag="rw")
            nc.vector.reciprocal(rw[:], wc[:])
            masks.make_identity(nc, identf[:])

            # ---------------- ACT feature builds ----------------
            # x features + y-half0 squares gate the forward start; y-half1
            # squares run under fwd-g0 (ACT has slack there)
            nc.scalar.square(xsq[:], xr[:])
            nc.scalar.mul(xm2[:], xr[:], -2.0)
            nc.sync.dma_start(X9[0:3, :], xsq[:])
            nc.sync.dma_start(X9[3:6, :], xm2[:])
            nc.scalar.square(ysq[:, 0:2048], yr[:, 0:2048])
            nc.scalar.dma_start(Y9[6:9, 0:2048], ysq[:, 0:2048])

            # ---------------- accumulators ----------------
            minf2 = spool.tile([128, 2 * NT], F32, tag="minf2")
            minb = spool.tile([128, MT], F32, tag="minb")
            fin = spool.tile([128, 3], F32, tag="fin")
            onescol = spool.tile([128, 1], F32, tag="onescol")
            nc.vector.memset(onescol[:], 1.0)
            sb_r = spool.tile([NT, 128], F32R, tag="sb_r")
            r_row = spool.tile([1, N], F32R, tag="r_row")

            def mm9(ps, lhsT, rhs_full, f0, fw):
                for k in range(fw // 512):
                    nc.tensor.matmul(ps[:, k * 512:(k + 1) * 512], lhsT,
                                     rhs_full[:, f0 + k * 512:f0 + (k + 1) * 512],
                                     start=True, stop=True)

            # ---------------- main loops ----------------
            with (
                tc.tile_pool(name="psum_main", bufs=4, space="PSUM") as mpool,
                tc.tile_pool(name="scratch", bufs=8) as scpool,
            ):
                def reduce_block(lhsT, rhs, f0, acc_col):
                    psQ = mpool.tile([128, 1024], F32, tag="d")
                    psP = mpool.tile([128, 1024], F32, tag="d")
                    mm9(psQ, lhsT, rhs, f0 + 1024, 1024)
                    sbQ = scpool.tile([128, 1024], F32, tag="sbq")
                    nc.scalar.copy(sbQ[:], psQ[:])
                    mm9(psP, lhsT, rhs, f0, 1024)
                    tout = scpool.tile([128, 1024], F32, tag="tout")
                    min_min_reduce(nc, tout[:], psP[:], sbQ[:], BIG, acc_col)

                # ---- forward g=0 (needs only Y9 half 0) ----
                for c in range(NT):
                    reduce_block(X9[:, c * 128:(c + 1) * 128], Y9,
                                 0, minf2[:, c:c + 1])
                    if c == 1:
                        # r-chain: transpose + r_row + R9 + XS9 (under fwd-g0)
                        ps_r = mpool.tile([NT, 128], F32, tag="d")
                        nc.tensor.transpose(ps_r[:], rw[:], identf[:])
                        nc.scalar.copy(sb_r[:], ps_r[:])
                        nc.sync.dma_start(r_row[:], sb_r[:])
                    if c == 3:
                        for half in range(2):
                            R9 = mpool.tile([9, 1024], F32, tag="d")
                            f0 = half * 1024
                            for k in range(2):
                                nc.tensor.matmul(
                                    R9[:, k * 512:(k + 1) * 512], ones1[:],
                                    r_row[:, f0 + k * 512:f0 + (k + 1) * 512],
                                    start=True, stop=True)
                            nc.vector.tensor_tensor(
                                XS9[:, f0:f0 + 1024],
                                X9[:, f0:f0 + 1024].bitcast(F32), R9[:],
                                op=MULT)
                    if c == 5:
                        # y-half1 squares under fwd-g0 (ACT)
                        nc.scalar.square(ysq[:, 2048:4096], yr[:, 2048:4096])
                        nc.scalar.dma_start(Y9[6:9, 2048:4096],
                                            ysq[:, 2048:4096])

                # ---- forward g=1 ----
                for c in range(NT):
                    reduce_block(X9[:, c * 128:(c + 1) * 128], Y9,
                                 2048, minf2[:, NT + c:NT + c + 1])

                # ---- backward ----
                for c in range(MT):
                    reduce_block(Y9[:, c * 128:(c + 1) * 128], XS9,
                                 0, minb[:, c:c + 1])

            # ---------------- finish ----------------
            minf = spool.tile([128, NT], F32, tag="minf")
            nc.vector.tensor_tensor(minf[:], minf2[:, 0:NT], minf2[:, NT:2 * NT],
                                    op=MIN)
            wm = spool.tile([128, NT], F32, tag="wm")
            nc.vector.scalar_tensor_tensor(wm[:], minf[:], 0.0, wN[:],
                                           op0=mybir.AluOpType.max, op1=MULT)
            nc.vector.tensor_scalar_max(minb[:], minb[:], 0.0)
            nc.vector.tensor_reduce(fin[:, 0:1], wm[:], axis=AX, op=ADD)
            nc.vector.tensor_reduce(fin[:, 1:2], wN[:], axis=AX, op=ADD)
            nc.vector.tensor_reduce(fin[:, 2:3], minb[:], axis=AX, op=ADD)

            with tc.tile_pool(name="psum_f", bufs=1, space="PSUM") as fps:
                ps3 = fps.tile([1, 3], F32, tag="ps3")
                nc.tensor.matmul(ps3[:], onescol[:], fin[:], start=True,
                                 stop=True)
                s3 = spool.tile([1, 3], F32, tag="s3")
                nc.vector.tensor_copy(s3[:], ps3[:])

            wsum = spool.tile([1, 1], F32, tag="wsum")
            nc.vector.tensor_scalar_max(wsum[:], s3[0:1, 1:2], CHAMFER_EPS)
            rwsum = spool.tile([1, 1], F32, tag="rwsum")
            nc.vector.reciprocal(rwsum[:], wsum[:])
            fwd = spool.tile([1, 1], F32, tag="fwd")
            nc.vector.tensor_tensor(fwd[:], s3[0:1, 0:1], rwsum[:], op=MULT)
            loss = spool.tile([1, 1], F32, tag="loss")
            nc.vector.scalar_tensor_tensor(loss[:], s3[0:1, 2:3], 1.0 / M,
                                           fwd[:], op0=MULT, op1=ADD)
            nc.sync.dma_start(out[:], loss[:])

    nc.compile()
    return nc


_NC_CACHE = {}


def get_nc():
    if "nc" not in _NC_CACHE:
        _NC_CACHE["nc"] = build_nc()
    return _NC_CACHE["nc"]


_ONES3 = np.ones((3, M), dtype=np.float32)


def make_in_maps(points, decoded_points, decoded_weights):
    in_maps = []
    for b in range(B):
        xT = np.ascontiguousarray(decoded_points[b].T).astype(np.float32)
        yT = np.ascontiguousarray(points[b].T).astype(np.float32)
        wT = np.ascontiguousarray(
            decoded_weights[b].reshape(NT, 128).T).astype(np.float32)
        in_maps.append({"xT": xT, "yT": yT, "wT": wT, "ones3": _ONES3})
    return in_maps


def kernel(points, decoded_points, decoded_weights):
    nc = get_nc()
    in_maps = make_in_maps(points, decoded_points, decoded_weights)
    res = run_bass_kernel_spmd(nc, in_maps, core_ids=list(range(B)))
    per_core = np.array([res.results[b]["loss"][0, 0] for b in range(B)],
                        dtype=np.float32)
    return np.asarray(per_core.mean(), dtype=np.float32)

